# revision 5
# baseline (speedup 1.0000x reference)
import sys

sys.path.insert(0, "/opt/trn_rl_repo")

import numpy as np

import concourse.bass as bass
import concourse.mybir as mybir
import concourse.tile as tile
from concourse import bacc, bass_utils

# Problem constants (nn_Generator moe_routing)
BATCH = 1024
ZDIM = 128
N_EXPERTS = 16
E_OUT = 3 * 64 * 64  # 12288 output features per expert
N_CORES = 8
EXP_PER_CORE = N_EXPERTS // N_CORES  # 2
OTILE = 512
N_OTILES = E_OUT // OTILE  # 24

_NC_CACHE = {}


def _build_nc(cap: int):
    """Per-core program: for 2 experts, out[e] = z_e @ W_e.T + b_e.

    Inputs (per core):
      zt   [ZDIM, 2*cap]   z rows for the core's experts, transposed
      wt   [ZDIM, 2*E_OUT] W.T slice for the core's two experts
      bv   [1, 2*E_OUT]    bias slice
      ones [1, 128]        all-ones row (bias broadcast via K=1 matmul)
    Output:
      out  [2*cap, E_OUT]
    """
    nc = bacc.Bacc(None, target_bir_lowering=False)
    zt = nc.dram_tensor("zt", [ZDIM, 2 * cap], mybir.dt.float32, kind="ExternalInput")
    wt = nc.dram_tensor("wt", [ZDIM, 2 * E_OUT], mybir.dt.float32, kind="ExternalInput")
    bv = nc.dram_tensor("bv", [1, 2 * E_OUT], mybir.dt.float32, kind="ExternalInput")
    ones = nc.dram_tensor("ones", [1, 128], mybir.dt.float32, kind="ExternalInput")
    out = nc.dram_tensor("out", [2 * cap, E_OUT], mybir.dt.float32, kind="ExternalOutput")

    with tile.TileContext(nc) as tc:
        with (
            tc.tile_pool(name="zpool", bufs=1) as zpool,
            tc.tile_pool(name="wpool", bufs=4) as wpool,
            tc.tile_pool(name="opool", bufs=4) as opool,
            tc.tile_pool(name="psum", bufs=4, space="PSUM") as psum_pool,
        ):
            ones_sb = zpool.tile([1, 128], mybir.dt.float32, tag="ones")
            nc.gpsimd.dma_start(out=ones_sb, in_=ones[:, :])
            b_sb = zpool.tile([1, 2 * E_OUT], mybir.dt.float32, tag="bias")
            nc.gpsimd.dma_start(out=b_sb, in_=bv[:, :])

            for e in range(EXP_PER_CORE):
                z_sb = zpool.tile([ZDIM, cap], mybir.dt.float32, tag=f"z{e}")
                nc.gpsimd.dma_start(out=z_sb, in_=zt[:, e * cap : (e + 1) * cap])
                for j in range(N_OTILES):
                    off = e * E_OUT + j * OTILE
                    w_sb = wpool.tile([ZDIM, OTILE], mybir.dt.float32)
                    nc.gpsimd.dma_start(out=w_sb, in_=wt[:, off : off + OTILE])
                    ps = psum_pool.tile([cap, OTILE], mybir.dt.float32)
                    nc.tensor.matmul(ps, z_sb, w_sb, start=True, stop=False)
                    nc.tensor.matmul(
                        ps,
                        ones_sb[:1, :cap],
                        b_sb[:1, off : off + OTILE],
                        start=False,
                        stop=True,
                    )
                    o_sb = opool.tile([cap, OTILE], mybir.dt.float32)
                    nc.vector.tensor_copy(o_sb, ps)
                    nc.gpsimd.dma_start(
                        out=out[e * cap : (e + 1) * cap, j * OTILE : (j + 1) * OTILE],
                        in_=o_sb,
                    )
    nc.compile()
    return nc


def kernel(z, c, W, b):
    z = np.asarray(z, dtype=np.float32)
    c_np = np.asarray(c).astype(np.int64)
    W = np.asarray(W, dtype=np.float32)
    b = np.asarray(b, dtype=np.float32)

    batch = z.shape[0]
    # Group sample indices by selected expert
    idx_per_e = [np.nonzero(c_np == e)[0] for e in range(N_EXPERTS)]
    counts = [len(ix) for ix in idx_per_e]
    cap = max(1, min(128, max(counts)))
    # round up to multiple of 16 for tidy DMA/partition shapes
    cap = min(128, ((cap + 15) // 16) * 16)

    WT = np.ascontiguousarray(W.T)  # [ZDIM, N_EXPERTS*E_OUT]
    ones = np.ones((1, 128), dtype=np.float32)

    in_maps = []
    for k in range(N_CORES):
        zt_k = np.zeros((ZDIM, EXP_PER_CORE * cap), dtype=np.float32)
        for i in range(EXP_PER_CORE):
            e = EXP_PER_CORE * k + i
            ix = idx_per_e[e][:cap]
            zt_k[:, i * cap : i * cap + len(ix)] = z[ix].T
        lo = EXP_PER_CORE * k * E_OUT
        hi = lo + EXP_PER_CORE * E_OUT
        in_maps.append(
            {
                "zt": zt_k,
                "wt": np.ascontiguousarray(WT[:, lo:hi]),
                "bv": np.ascontiguousarray(b[lo:hi]).reshape(1, -1),
                "ones": ones,
            }
        )

    global _LAST_IN_MAPS
    _LAST_IN_MAPS = in_maps
    if cap not in _NC_CACHE:
        _NC_CACHE[cap] = _build_nc(cap)
    res = bass_utils.run_bass_kernel_spmd(_NC_CACHE[cap], in_maps, list(range(N_CORES)))

    out = np.empty((batch, E_OUT), dtype=np.float32)
    for k in range(N_CORES):
        o_k = np.asarray(res.results[k]["out"])
        for i in range(EXP_PER_CORE):
            e = EXP_PER_CORE * k + i
            ix = idx_per_e[e][:cap]
            out[ix] = o_k[i * cap : i * cap + len(ix)]
            # overflow samples (expert count > cap) computed on host; with
            # uniform routing this never triggers, but keeps kernel correct
            for s in idx_per_e[e][cap:]:
                out[s] = z[s] @ W[e * E_OUT : (e + 1) * E_OUT].T + b[e * E_OUT : (e + 1) * E_OUT]
    return out.reshape(batch, 3, 64, 64)
